# revision 15
# baseline (speedup 1.0000x reference)
"""Trainium2 Bass kernel for nn_MultiHeadAttention_56118042690041.

8-core sharding: batch x heads tensor-parallel.
  core c (0..7): batch b = c//4, heads 4*(c%4) .. 4*(c%4)+4 (as 2 packed pairs).
Per core:
  - QKV projections for its 4 heads (head-pairs packed to M=128), contraction
    over D in PSUM, fp32r matmuls.
  - Attention per head (note reference's faithful "bug": scores = v2 @ k2^T,
    weighted sum of q2): scoresT[t,s] tiles on PE (K=dk=64), exp on ACT
    (no max-subtraction needed: scores are tiny by construction), AV matmul
    with a ones-column augmented q2 giving the softmax denominator for free,
    normalization on DVE.
  - Head outputs (headoutT layout [dk, s]) AllGather'd across the 4 cores of
    the same batch group -> full [H*DK, S] per core.
  - Output projection: each core computes a disjoint 256-wide d-slice of
    out = headout @ Wo^T + bo (column-sharded Wo -> SPMD-uniform program).
Host: slices weights per core, transposes x, concatenates disjoint outputs.
"""

import contextlib
import ctypes
import os
import sys
import types

import numpy as np

if "/opt/trn_rl_repo" not in sys.path:
    sys.path.insert(0, "/opt/trn_rl_repo")

# ---------------------------------------------------------------- shims ----


def _install_antenv_shim():
    """Provide antenv.axon_hooks (NTFF profile hook) if the image lacks it."""
    try:
        import antenv.axon_hooks  # noqa: F401

        return
    except ImportError:
        pass

    def _hook_factory():
        so_path = "/opt/axon/libaxon_pjrt.so"
        try:
            lib = ctypes.CDLL(so_path)
        except OSError:
            return None
        if not hasattr(lib, "axon_start_nrt_profile"):
            return None
        lib.axon_start_nrt_profile.argtypes = [
            ctypes.POINTER(ctypes.c_int64),
            ctypes.c_size_t,
        ]
        lib.axon_start_nrt_profile.restype = ctypes.c_int64
        lib.axon_stop_nrt_profile.argtypes = [ctypes.c_char_p]
        lib.axon_stop_nrt_profile.restype = ctypes.c_int64

        @contextlib.contextmanager
        def _hook(output_dir, device_ids):
            import jax

            jax.devices()
            if device_ids:
                ids = (ctypes.c_int64 * len(device_ids))(*device_ids)
                rc = lib.axon_start_nrt_profile(ids, len(device_ids))
            else:
                rc = lib.axon_start_nrt_profile(None, 0)
            if rc != 0:
                raise RuntimeError(f"axon_start_nrt_profile rc={rc}")
            try:
                yield
            finally:
                n = lib.axon_stop_nrt_profile(str(output_dir).encode())
                print(f"ntff profile: {n} file(s) -> {output_dir}", file=sys.stderr)

        return _hook

    hook = _hook_factory()
    mod = types.ModuleType("antenv.axon_hooks")
    mod.get_axon_ntff_profile_hook = lambda: hook
    mod.set_axon_ntff_profile_hook = lambda h: None
    sys.modules["antenv.axon_hooks"] = mod


def _install_tile_drain_patch():
    """This walrus build rejects >1 sync wait on the Tile tail Drain; split the
    waits across chained single-wait drains."""
    import concourse.tile as tile

    if getattr(tile.TileContext, "_drain_patch_installed", False):
        return

    def _drain_and_barrier(self, tick_clock, wait_clock):
        nc = self.nc
        drain_inst = nc.sync.drain()
        wait_clock.add_sem_waits(
            drain_inst.ins, tile.ScopedClock({None: tick_clock.global_clock})
        )
        si = drain_inst.ins.sync_info
        waits = list(si.on_wait) if si is not None and si.on_wait else []
        if len(waits) > 1:
            si.on_wait = waits[:1]
            assert self.sems is not None
            by_num = {h.num: h for h in self.sems.allocated().values()}
            for w in waits[1:]:
                d2 = nc.sync.drain()
                h = by_num.get(w.id)
                assert h is not None, f"no sem handle for wait {w.ant_name}"
                d2.wait_op(h, w.wait_value, "sem-ge", check=False)
        nc.all_engine_barrier()
        assert self.sems is not None
        popped = nc._tile_sem_poison_stack.pop()
        assert popped is self._sem_poison
        nc.clear_and_free_semaphores(list(self.sems.allocated().values()))
        nc.all_engine_barrier()

    tile.TileContext._drain_and_barrier = _drain_and_barrier
    tile.TileContext._drain_patch_installed = True


_install_antenv_shim()


def _split_multi_waits(nc, max_waits=1):
    """This walrus build rejects instructions carrying more than ~1 sync wait.
    Move excess waits onto same-engine NOPs inserted immediately before the
    instruction (sequencer waits execute in stream order, so this is
    semantics-preserving)."""
    import bass_rust
    import concourse.mybir as mybir

    n = 0
    for bb in nc.m.functions[0].blocks:
        insts = bb.instructions
        out = []
        for inst in insts:
            si = inst.sync_info
            waits = list(si.on_wait) if si is not None and si.on_wait else []
            if len(waits) > max_waits:
                keep = waits[-max_waits:]
                for w in waits[:-max_waits]:
                    nop = mybir.InstNoOp(name=f"waitnop_{n}", ins=[], outs=[])
                    n += 1
                    nop.engine = inst.engine
                    nop.sync_info = bass_rust.SyncInfo(on_wait=[w], on_update=[])
                    out.append(nop)
                si.on_wait = keep
            out.append(inst)
        if len(out) != len(insts):
            insts[:] = out
    return n


# ------------------------------------------------------------- program -----

N_CORES = 8
GROUP = 4  # cores per batch group
USE_FP32R = True

last_results = None  # BassKernelResults of the most recent run (for test.py)


def build_program(S=2048, DM=1024, H=16, DK=64, use_fp32r=USE_FP32R, split_waits=True):
    """Emit the SPMD Bass/Tile program. Returns nc."""
    import concourse.bass as bass
    import concourse.mybir as mybir
    import concourse.tile as tile

    _install_tile_drain_patch()

    f32 = mybir.dt.float32
    f32r = mybir.dt.float32r
    NPAIR = 2  # head pairs per core (4 heads)
    KT = DM // 128  # contraction tiles for projections
    TT = S // 128  # t tiles (scores row blocks / AV contraction tiles)
    SQ = min(1024, S)  # scores/exp free width
    NSH = S // SQ
    SB = min(512, SQ)  # AV / normalize block
    NSB2 = SQ // SB
    MMN = min(512, S)  # matmul moving max (fp32)
    HDK = H * DK  # concat dim (1024)
    KO = HDK // 128  # outproj contraction tiles
    DSL = HDK // GROUP  # out d-slice per core (256)
    OSB = S // 128  # outproj s blocks

    nc = bass.Bass(
        trn_type="TRN2", target_bir_lowering=False, debug=False, num_devices=N_CORES
    )

    def din(name, shape):
        return nc.dram_tensor(name, shape, f32, kind="ExternalInput").ap()

    xT = {p: din(f"x{p}T", [DM, S]) for p in ("q", "k", "v")}  # x[b].T per kind
    W = {p: din(f"w{p}", [NPAIR, DM, 128]) for p in ("q", "k", "v")}  # pair-packed W.T
    bq = din("bq", [NPAIR, 128, 1])
    bk8 = din("bk8", [NPAIR, 128, 1])  # bk / sqrt(dk)
    bv = din("bv", [NPAIR, 128, 1])
    woT = din("woT", [HDK, DSL])  # Wo.T columns for this core's d-slice
    bo_bc = din("bo_bc", [128, DSL])  # bo d-slice broadcast to 128 partitions
    ident = din("ident", [128, 64])  # eye(64) stacked twice (both partition halves)
    out_ap = nc.dram_tensor("out", [S, DSL], f32, kind="ExternalOutput").ap()

    fr = f32r if use_fp32r else f32  # dtype for matmul operand tiles

    with tile.TileContext(nc) as tc:
        with contextlib.ExitStack() as ctx:
            sb = ctx.enter_context(tc.tile_pool(name="sb", bufs=2))
            big = ctx.enter_context(tc.tile_pool(name="big", bufs=8))
            ps = ctx.enter_context(tc.tile_pool(name="ps", bufs=2, space="PSUM"))
            dram = ctx.enter_context(tc.tile_pool(name="dram", bufs=1, space="DRAM"))

            # --- constants / small tiles ---
            ident_sb = sb.tile([128, 64], f32, tag="ident", bufs=1)
            nc.sync.dma_start(ident_sb[:], ident[:])
            ones64 = sb.tile([1, 64], f32, tag="ones", bufs=1)
            nc.gpsimd.memset(ones64[:], 1.0)
            ones128 = sb.tile([128, 1], f32, tag="ones1", bufs=1)
            nc.gpsimd.memset(ones128[:], 1.0)
            bq_sb = sb.tile([128, NPAIR], f32, tag="bq", bufs=1)
            bk_sb = sb.tile([128, NPAIR], f32, tag="bk", bufs=1)
            bv_sb = sb.tile([128, NPAIR], f32, tag="bv", bufs=1)
            for p in range(NPAIR):
                nc.sync.dma_start(bq_sb[:, p : p + 1], bq[p])
                nc.sync.dma_start(bk_sb[:, p : p + 1], bk8[p])
                nc.sync.dma_start(bv_sb[:, p : p + 1], bv[p])
            bo_sb = sb.tile([128, DSL], f32, tag="bo", bufs=1)
            nc.sync.dma_start(bo_sb[:], bo_bc[:])
            woT_sb = sb.tile([128, KO * DSL], fr, tag="wo", bufs=1)
            for k in range(KO):
                nc.sync.dma_start(
                    woT_sb[:, k * DSL : (k + 1) * DSL],
                    woT[k * 128 : (k + 1) * 128, :].bitcast(fr),
                )

            # --- phase P: projections -> q2T/k2T/v2T pair tiles [128, S] ---
            proj_out = {}
            for kind, bias_kind in (("v", "v"), ("k", "k"), ("q", "q")):
                w_sb = [
                    sb.tile([128, KT * 128], fr, tag="w", bufs=4, name=f"w_{kind}{p}")
                    for p in range(NPAIR)
                ]
                for p in range(NPAIR):
                    for k in range(KT):
                        nc.sync.dma_start(
                            w_sb[p][:, k * 128 : (k + 1) * 128],
                            W[kind][p, k * 128 : (k + 1) * 128, :].bitcast(fr),
                        )
                odt = f32 if kind == "q" else fr
                outs = [
                    big.tile([128, S], odt, tag="big2048", name=f"{kind}2T_{p}")
                    for p in range(NPAIR)
                ]
                proj_out[kind] = outs
                NTS = S // SQ  # proj t-slices (reuse SQ width)
                for ts in range(NTS):
                    prs = [
                        ps.tile([128, SQ], f32, tag="big", name=f"pr{kind}{ts}_{p}")
                        for p in range(NPAIR)
                    ]
                    for k in range(KT):
                        xt = sb.tile([128, SQ], fr, tag="xt", bufs=3, name=f"xt{kind}")
                        nc.sync.dma_start(
                            xt[:],
                            xT[kind][
                                k * 128 : (k + 1) * 128, ts * SQ : (ts + 1) * SQ
                            ].bitcast(fr),
                        )
                        for p in range(NPAIR):
                            for j in range(SQ // MMN):
                                nc.tensor.matmul(
                                    prs[p][:, j * MMN : (j + 1) * MMN],
                                    (w_sb[p][:, k * 128 : (k + 1) * 128]),
                                    (xt[:, j * MMN : (j + 1) * MMN]),
                                    start=(k == 0),
                                    stop=(k == KT - 1),
                                )
                    for p in range(NPAIR):
                        dst = outs[p][:, ts * SQ : (ts + 1) * SQ]
                        if kind == "q":
                            nc.vector.tensor_copy(dst, prs[p][:])
                        elif kind == "k":
                            nc.vector.tensor_scalar(
                                dst,
                                prs[p][:],
                                1.0 / 8.0,
                                bk_sb[:, p : p + 1],
                                mybir.AluOpType.mult,
                                mybir.AluOpType.add,
                            )
                        else:
                            nc.vector.tensor_scalar_add(
                                dst, prs[p][:], bv_sb[:, p : p + 1]
                            )
            q2T, k2T, v2T = proj_out["q"], proj_out["k"], proj_out["v"]

            # --- phase T: q2 transpose -> q2aug [t, dk|1] per head ---
            q2aug = []
            for h in range(2 * NPAIR):
                p, prow = h // 2, 64 * (h % 2)
                qa = big.tile([128, TT * 65], fr, tag="q2aug", bufs=2 * NPAIR)
                q2aug.append(qa)
                for t in range(TT):
                    nc.vector.tensor_copy(qa[:, t * 65 + 64 : t * 65 + 65], ones128[:])
                for t in range(TT):
                    tr = ps.tile([128, 64], f32, tag="sm", name="tr")
                    nc.tensor.transpose(
                        tr[:],
                        q2T[p][prow : prow + 64, t * 128 : (t + 1) * 128],
                        ident_sb[prow : prow + 64, :],
                    )
                    nc.vector.tensor_copy(qa[:, t * 65 : t * 65 + 64], tr[:])

            # --- phase A: attention per head ---
            headout = [
                big.tile([128, S], f32, tag="big2048", name=f"headout_{p}")
                for p in range(NPAIR)
            ]
            for h in range(2 * NPAIR):
                p, prow = h // 2, 64 * (h % 2)
                for sh in range(NSH):
                    expt = []
                    for tb in range(TT):
                        sc = ps.tile([128, SQ], f32, tag="big", name="sc")
                        for j in range(SQ // MMN):
                            nc.tensor.matmul(
                                sc[:, j * MMN : (j + 1) * MMN],
                                (k2T[p][prow : prow + 64, tb * 128 : (tb + 1) * 128]),
                                (
                                    v2T[p][
                                        prow : prow + 64,
                                        sh * SQ + j * MMN : sh * SQ + (j + 1) * MMN,
                                    ]
                                ),
                                start=True,
                                stop=True,
                            )
                        et = sb.tile([128, SQ], fr, tag="expt", bufs=TT, name="et")
                        nc.scalar.activation(
                            et[:], sc[:], mybir.ActivationFunctionType.Exp
                        )
                        expt.append(et)
                    for s2 in range(NSB2):
                        av = ps.tile([65, SB], f32, tag="av", name="av")
                        for tk in range(TT):
                            nc.tensor.matmul(
                                av[:],
                                (q2aug[h][:, tk * 65 : tk * 65 + 65]),
                                (expt[tk][:, s2 * SB : (s2 + 1) * SB]),
                                start=(tk == 0),
                                stop=(tk == TT - 1),
                            )
                        recip = sb.tile([1, SB], f32, tag="recip", bufs=2)
                        nc.vector.reciprocal(recip[:], av[64:65, :])
                        bc = ps.tile([64, SB], f32, tag="sm", name="bc")
                        nc.tensor.matmul(
                            bc[:], (ones64[:]), (recip[:]), start=True, stop=True
                        )
                        bcs = sb.tile([64, SB], f32, tag="bcs", bufs=2, name="bcs")
                        nc.vector.tensor_copy(bcs[:], bc[:])
                        dst = headout[p][
                            prow : prow + 64, sh * SQ + s2 * SB : sh * SQ + (s2 + 1) * SB
                        ]
                        nc.vector.tensor_mul(dst, av[0:64, :], bcs[:])
                        nc.vector.tensor_scalar_add(
                            dst, dst, bq_sb[prow : prow + 64, p : p + 1]
                        )

            # --- phase G: AllGather head outputs within batch group ---
            cc_in = dram.tile([NPAIR * 128, S], f32, name="cc_in")
            for p in range(NPAIR):
                nc.sync.dma_start(cc_in[p * 128 : (p + 1) * 128, :], headout[p][:])
            cc_out = dram.tile([HDK, S], f32, name="cc_out")
            nc.gpsimd.collective_compute(
                "AllGather",
                mybir.AluOpType.bypass,
                replica_groups=[[0, 1, 2, 3], [4, 5, 6, 7]],
                ins=[cc_in.opt()],
                outs=[cc_out.opt()],
            )

            # --- phase O: output projection (d-slice) ---
            for si in range(OSB):
                po = ps.tile([128, DSL], f32, tag="big", name="po")
                for k in range(KO):
                    ch = sb.tile([128, 128], fr, tag="ch", bufs=3, name="ch")
                    nc.sync.dma_start(
                        ch[:],
                        cc_out[
                            k * 128 : (k + 1) * 128, si * 128 : (si + 1) * 128
                        ].bitcast(fr),
                    )
                    nc.tensor.matmul(
                        po[:],
                        (ch[:]),
                        (woT_sb[:, k * DSL : (k + 1) * DSL]),
                        start=(k == 0),
                        stop=(k == KO - 1),
                    )
                ob = sb.tile([128, DSL], f32, tag="ob", bufs=3, name="ob")
                nc.vector.tensor_add(ob[:], po[:], bo_sb[:])
                nc.sync.dma_start(out_ap[si * 128 : (si + 1) * 128, :], ob[:])

    if split_waits:
        _split_multi_waits(nc)
    return nc


def make_in_maps(v, k, q, Wq, bqv, Wk, bkv, Wv, bvv, Wo, bov, S, DM, H, DK):
    """Per-core input dicts from full inputs (all host-side prep is slicing /
    transpose / trivial broadcast)."""
    HDK = H * DK
    DSL = HDK // GROUP
    xT = {}
    for b in range(2):
        xT[("q", b)] = np.ascontiguousarray(q[b].T)  # [DM, S]
        xT[("k", b)] = np.ascontiguousarray(k[b].T)
        xT[("v", b)] = np.ascontiguousarray(v[b].T)
    WoT = np.ascontiguousarray(Wo.T)  # [HDK, HDK_out]
    ident = np.vstack([np.eye(64, dtype=np.float32)] * 2)
    in_maps = []
    for c in range(N_CORES):
        b = c // GROUP
        h0 = 4 * (c % GROUP)
        m = {
            "xqT": xT[("q", b)],
            "xkT": xT[("k", b)],
            "xvT": xT[("v", b)],
            "ident": ident,
        }
        for kind, Wt, bt in (("q", Wq, bqv), ("k", Wk, bkv), ("v", Wv, bvv)):
            wp = np.empty((2, DM, 128), np.float32)
            bp = np.empty((2, 128, 1), np.float32)
            for p in range(2):
                ha, hb = h0 + 2 * p, h0 + 2 * p + 1
                wp[p, :, :64] = Wt[ha].T
                wp[p, :, 64:] = Wt[hb].T
                bp[p, :64, 0] = bt[ha]
                bp[p, 64:, 0] = bt[hb]
            m[f"w{kind}"] = wp
            if kind == "q":
                m["bq"] = bp
            elif kind == "k":
                m["bk8"] = bp / 8.0
            else:
                m["bv"] = bp
        d0 = DSL * (c % GROUP)
        m["woT"] = np.ascontiguousarray(WoT[:, d0 : d0 + DSL])
        m["bo_bc"] = np.ascontiguousarray(
            np.broadcast_to(bov[d0 : d0 + DSL], (128, DSL))
        )
        in_maps.append(m)
    return in_maps


def kernel(v, k, q, Wq, bq, Wk, bk, Wv, bv, Wo, bo, _trace=False):
    """Full inputs in, full output out. Runs the SPMD Bass kernel on 8 cores."""
    global last_results
    from concourse.bass_utils import run_bass_kernel_spmd

    v, k, q = (np.asarray(a, np.float32) for a in (v, k, q))
    B, S, DM = q.shape
    H, DK = Wq.shape[0], Wq.shape[1]
    HDK = H * DK
    DSL = HDK // GROUP

    nc = build_program(S=S, DM=DM, H=H, DK=DK)
    in_maps = make_in_maps(
        np.asarray(v, np.float32),
        np.asarray(k, np.float32),
        np.asarray(q, np.float32),
        *(np.asarray(a, np.float32) for a in (Wq, bq, Wk, bk, Wv, bv, Wo, bo)),
        S=S,
        DM=DM,
        H=H,
        DK=DK,
    )
    res = run_bass_kernel_spmd(
        nc, in_maps, list(range(N_CORES)), trace=_trace
    )
    last_results = res
    out = np.empty((B, S, HDK), np.float32)
    for c in range(N_CORES):
        b = c // GROUP
        d0 = DSL * (c % GROUP)
        out[b, :, d0 : d0 + DSL] = res.results[c]["out"]
    return out


# revision 16
# speedup vs baseline: 1.0921x; 1.0921x over previous
"""Trainium2 Bass kernel for nn_MultiHeadAttention_56118042690041.

8-core sharding: batch x heads tensor-parallel.
  core c (0..7): batch b = c//4, heads 4*(c%4) .. 4*(c%4)+4 (as 2 packed pairs).
Per core:
  - QKV projections for its 4 heads (head-pairs packed to M=128), contraction
    over D in PSUM, fp32r matmuls.
  - Attention per head (note reference's faithful "bug": scores = v2 @ k2^T,
    weighted sum of q2): scoresT[t,s] tiles on PE (K=dk=64), exp on ACT
    (no max-subtraction needed: scores are tiny by construction), AV matmul
    with a ones-column augmented q2 giving the softmax denominator for free,
    normalization on DVE.
  - Head outputs (headoutT layout [dk, s]) AllGather'd across the 4 cores of
    the same batch group -> full [H*DK, S] per core.
  - Output projection: each core computes a disjoint 256-wide d-slice of
    out = headout @ Wo^T + bo (column-sharded Wo -> SPMD-uniform program).
Host: slices weights per core, transposes x, concatenates disjoint outputs.
"""

import contextlib
import ctypes
import os
import sys
import types

import numpy as np

if "/opt/trn_rl_repo" not in sys.path:
    sys.path.insert(0, "/opt/trn_rl_repo")

# ---------------------------------------------------------------- shims ----


def _install_antenv_shim():
    """Provide antenv.axon_hooks (NTFF profile hook) if the image lacks it."""
    try:
        import antenv.axon_hooks  # noqa: F401

        return
    except ImportError:
        pass

    def _hook_factory():
        so_path = "/opt/axon/libaxon_pjrt.so"
        try:
            lib = ctypes.CDLL(so_path)
        except OSError:
            return None
        if not hasattr(lib, "axon_start_nrt_profile"):
            return None
        lib.axon_start_nrt_profile.argtypes = [
            ctypes.POINTER(ctypes.c_int64),
            ctypes.c_size_t,
        ]
        lib.axon_start_nrt_profile.restype = ctypes.c_int64
        lib.axon_stop_nrt_profile.argtypes = [ctypes.c_char_p]
        lib.axon_stop_nrt_profile.restype = ctypes.c_int64

        @contextlib.contextmanager
        def _hook(output_dir, device_ids):
            import jax

            jax.devices()
            if device_ids:
                ids = (ctypes.c_int64 * len(device_ids))(*device_ids)
                rc = lib.axon_start_nrt_profile(ids, len(device_ids))
            else:
                rc = lib.axon_start_nrt_profile(None, 0)
            if rc != 0:
                raise RuntimeError(f"axon_start_nrt_profile rc={rc}")
            try:
                yield
            finally:
                n = lib.axon_stop_nrt_profile(str(output_dir).encode())
                print(f"ntff profile: {n} file(s) -> {output_dir}", file=sys.stderr)

        return _hook

    hook = _hook_factory()
    mod = types.ModuleType("antenv.axon_hooks")
    mod.get_axon_ntff_profile_hook = lambda: hook
    mod.set_axon_ntff_profile_hook = lambda h: None
    sys.modules["antenv.axon_hooks"] = mod


def _install_tile_drain_patch():
    """This walrus build rejects >1 sync wait on the Tile tail Drain; split the
    waits across chained single-wait drains."""
    import concourse.tile as tile

    if getattr(tile.TileContext, "_drain_patch_installed", False):
        return

    def _drain_and_barrier(self, tick_clock, wait_clock):
        nc = self.nc
        drain_inst = nc.sync.drain()
        wait_clock.add_sem_waits(
            drain_inst.ins, tile.ScopedClock({None: tick_clock.global_clock})
        )
        si = drain_inst.ins.sync_info
        waits = list(si.on_wait) if si is not None and si.on_wait else []
        if len(waits) > 1:
            si.on_wait = waits[:1]
            assert self.sems is not None
            by_num = {h.num: h for h in self.sems.allocated().values()}
            for w in waits[1:]:
                d2 = nc.sync.drain()
                h = by_num.get(w.id)
                assert h is not None, f"no sem handle for wait {w.ant_name}"
                d2.wait_op(h, w.wait_value, "sem-ge", check=False)
        nc.all_engine_barrier()
        assert self.sems is not None
        popped = nc._tile_sem_poison_stack.pop()
        assert popped is self._sem_poison
        nc.clear_and_free_semaphores(list(self.sems.allocated().values()))
        nc.all_engine_barrier()

    tile.TileContext._drain_and_barrier = _drain_and_barrier
    tile.TileContext._drain_patch_installed = True


_install_antenv_shim()


def _split_multi_waits(nc, max_waits=1):
    """This walrus build rejects instructions carrying more than ~1 sync wait.
    Move excess waits onto same-engine NOPs inserted immediately before the
    instruction (sequencer waits execute in stream order, so this is
    semantics-preserving)."""
    import bass_rust
    import concourse.mybir as mybir

    n = 0
    for bb in nc.m.functions[0].blocks:
        insts = bb.instructions
        out = []
        for inst in insts:
            si = inst.sync_info
            waits = list(si.on_wait) if si is not None and si.on_wait else []
            if len(waits) > max_waits:
                keep = waits[-max_waits:]
                for w in waits[:-max_waits]:
                    nop = mybir.InstNoOp(name=f"waitnop_{n}", ins=[], outs=[])
                    n += 1
                    nop.engine = inst.engine
                    nop.sync_info = bass_rust.SyncInfo(on_wait=[w], on_update=[])
                    out.append(nop)
                si.on_wait = keep
            out.append(inst)
        if len(out) != len(insts):
            insts[:] = out
    return n


# ------------------------------------------------------------- program -----

N_CORES = 8
GROUP = 4  # cores per batch group
USE_FP32R = True
ATTN_BF16 = True

last_results = None  # BassKernelResults of the most recent run (for test.py)


def build_program(S=2048, DM=1024, H=16, DK=64, use_fp32r=USE_FP32R, attn_bf16=ATTN_BF16, split_waits=True):
    """Emit the SPMD Bass/Tile program. Returns nc."""
    import concourse.bass as bass
    import concourse.mybir as mybir
    import concourse.tile as tile

    _install_tile_drain_patch()

    f32 = mybir.dt.float32
    f32r = mybir.dt.float32r
    NPAIR = 2  # head pairs per core (4 heads)
    KT = DM // 128  # contraction tiles for projections
    TT = S // 128  # t tiles (scores row blocks / AV contraction tiles)
    SQ = min(1024, S)  # scores/exp free width
    NSH = S // SQ
    SB = min(512, SQ)  # AV / normalize block
    NSB2 = SQ // SB
    MMN = min(512, S)  # matmul moving max (fp32)
    HDK = H * DK  # concat dim (1024)
    KO = HDK // 128  # outproj contraction tiles
    DSL = HDK // GROUP  # out d-slice per core (256)
    OSB = S // 128  # outproj s blocks

    nc = bass.Bass(
        trn_type="TRN2", target_bir_lowering=False, debug=False, num_devices=N_CORES
    )

    def din(name, shape):
        return nc.dram_tensor(name, shape, f32, kind="ExternalInput").ap()

    xT = {p: din(f"x{p}T", [DM, S]) for p in ("q", "k", "v")}  # x[b].T per kind
    W = {p: din(f"w{p}", [NPAIR, DM, 128]) for p in ("q", "k", "v")}  # pair-packed W.T
    bq = din("bq", [NPAIR, 128, 1])
    bk8 = din("bk8", [NPAIR, 128, 1])  # bk / sqrt(dk)
    bv = din("bv", [NPAIR, 128, 1])
    woT = din("woT", [HDK, DSL])  # Wo.T columns for this core's d-slice
    bo_bc = din("bo_bc", [128, DSL])  # bo d-slice broadcast to 128 partitions
    ident = din("ident", [128, 64])  # eye(64) stacked twice (both partition halves)
    out_ap = nc.dram_tensor("out", [S, DSL], f32, kind="ExternalOutput").ap()

    fr = f32r if use_fp32r else f32  # dtype for matmul operand tiles
    fa = mybir.dt.bfloat16 if attn_bf16 else fr  # attention matmul operand dtype

    with tile.TileContext(nc) as tc:
        with contextlib.ExitStack() as ctx:
            sb = ctx.enter_context(tc.tile_pool(name="sb", bufs=2))
            big = ctx.enter_context(tc.tile_pool(name="big", bufs=8))
            ps = ctx.enter_context(tc.tile_pool(name="ps", bufs=2, space="PSUM"))
            dram = ctx.enter_context(tc.tile_pool(name="dram", bufs=1, space="DRAM"))

            # --- constants / small tiles ---
            ident_sb = sb.tile([128, 64], f32, tag="ident", bufs=1)
            nc.sync.dma_start(ident_sb[:], ident[:])
            ones64 = sb.tile([1, 64], f32, tag="ones", bufs=1)
            nc.gpsimd.memset(ones64[:], 1.0)
            ones128 = sb.tile([128, 1], f32, tag="ones1", bufs=1)
            nc.gpsimd.memset(ones128[:], 1.0)
            bq_sb = sb.tile([128, NPAIR], f32, tag="bq", bufs=1)
            bk_sb = sb.tile([128, NPAIR], f32, tag="bk", bufs=1)
            bv_sb = sb.tile([128, NPAIR], f32, tag="bv", bufs=1)
            for p in range(NPAIR):
                nc.sync.dma_start(bq_sb[:, p : p + 1], bq[p])
                nc.sync.dma_start(bk_sb[:, p : p + 1], bk8[p])
                nc.sync.dma_start(bv_sb[:, p : p + 1], bv[p])
            bo_sb = sb.tile([128, DSL], f32, tag="bo", bufs=1)
            nc.sync.dma_start(bo_sb[:], bo_bc[:])
            woT_sb = sb.tile([128, KO * DSL], fr, tag="wo", bufs=1)
            for k in range(KO):
                nc.sync.dma_start(
                    woT_sb[:, k * DSL : (k + 1) * DSL],
                    woT[k * 128 : (k + 1) * 128, :].bitcast(fr),
                )

            # --- phase P: projections -> q2T/k2T/v2T pair tiles [128, S] ---
            proj_out = {}
            for kind, bias_kind in (("v", "v"), ("k", "k"), ("q", "q")):
                w_sb = [
                    sb.tile([128, KT * 128], fr, tag="w", bufs=4, name=f"w_{kind}{p}")
                    for p in range(NPAIR)
                ]
                for p in range(NPAIR):
                    for k in range(KT):
                        nc.sync.dma_start(
                            w_sb[p][:, k * 128 : (k + 1) * 128],
                            W[kind][p, k * 128 : (k + 1) * 128, :].bitcast(fr),
                        )
                odt = f32 if kind == "q" else fa
                otag = "big2048" if kind == "q" else "big2048h"
                outs = [
                    big.tile([128, S], odt, tag=otag, bufs=4, name=f"{kind}2T_{p}")
                    for p in range(NPAIR)
                ]
                proj_out[kind] = outs
                NTS = S // SQ  # proj t-slices (reuse SQ width)
                for ts in range(NTS):
                    prs = [
                        ps.tile([128, SQ], f32, tag="big", name=f"pr{kind}{ts}_{p}")
                        for p in range(NPAIR)
                    ]
                    for k in range(KT):
                        xt = sb.tile([128, SQ], fr, tag="xt", bufs=3, name=f"xt{kind}")
                        nc.sync.dma_start(
                            xt[:],
                            xT[kind][
                                k * 128 : (k + 1) * 128, ts * SQ : (ts + 1) * SQ
                            ].bitcast(fr),
                        )
                        for p in range(NPAIR):
                            for j in range(SQ // MMN):
                                nc.tensor.matmul(
                                    prs[p][:, j * MMN : (j + 1) * MMN],
                                    (w_sb[p][:, k * 128 : (k + 1) * 128]),
                                    (xt[:, j * MMN : (j + 1) * MMN]),
                                    start=(k == 0),
                                    stop=(k == KT - 1),
                                )
                    for p in range(NPAIR):
                        dst = outs[p][:, ts * SQ : (ts + 1) * SQ]
                        if kind == "q":
                            nc.vector.tensor_copy(dst, prs[p][:])
                        elif kind == "k":
                            nc.vector.tensor_scalar(
                                dst,
                                prs[p][:],
                                1.0 / 8.0,
                                bk_sb[:, p : p + 1],
                                mybir.AluOpType.mult,
                                mybir.AluOpType.add,
                            )
                        else:
                            nc.vector.tensor_scalar_add(
                                dst, prs[p][:], bv_sb[:, p : p + 1]
                            )
            q2T, k2T, v2T = proj_out["q"], proj_out["k"], proj_out["v"]

            # --- phase T: q2 transpose -> q2aug [t, dk|1] per head ---
            q2aug = []
            for h in range(2 * NPAIR):
                p, prow = h // 2, 64 * (h % 2)
                qa = big.tile([128, TT * 65], fa, tag="q2aug", bufs=2 * NPAIR)
                q2aug.append(qa)
                for t in range(TT):
                    nc.vector.tensor_copy(qa[:, t * 65 + 64 : t * 65 + 65], ones128[:])
                for t in range(TT):
                    tr = ps.tile([128, 64], f32, tag="sm", name="tr")
                    nc.tensor.transpose(
                        tr[:],
                        q2T[p][prow : prow + 64, t * 128 : (t + 1) * 128],
                        ident_sb[prow : prow + 64, :],
                    )
                    nc.vector.tensor_copy(qa[:, t * 65 : t * 65 + 64], tr[:])

            # --- phase A: attention per head ---
            headout = [
                big.tile([128, S], f32, tag="big2048", bufs=4, name=f"headout_{p}")
                for p in range(NPAIR)
            ]
            for h in range(2 * NPAIR):
                p, prow = h // 2, 64 * (h % 2)
                for sh in range(NSH):
                    expt = []
                    for tb in range(TT):
                        sc = ps.tile([128, SQ], f32, tag="big", name="sc")
                        for j in range(SQ // MMN):
                            nc.tensor.matmul(
                                sc[:, j * MMN : (j + 1) * MMN],
                                (k2T[p][prow : prow + 64, tb * 128 : (tb + 1) * 128]),
                                (
                                    v2T[p][
                                        prow : prow + 64,
                                        sh * SQ + j * MMN : sh * SQ + (j + 1) * MMN,
                                    ]
                                ),
                                start=True,
                                stop=True,
                            )
                        et = sb.tile([128, SQ], fa, tag="expt", bufs=min(TT + 8, 2 * TT), name="et")
                        nc.scalar.activation(
                            et[:], sc[:], mybir.ActivationFunctionType.Exp
                        )
                        expt.append(et)
                    for s2 in range(NSB2):
                        av = ps.tile([65, SB], f32, tag="av", name="av")
                        for tk in range(TT):
                            nc.tensor.matmul(
                                av[:],
                                (q2aug[h][:, tk * 65 : tk * 65 + 65]),
                                (expt[tk][:, s2 * SB : (s2 + 1) * SB]),
                                start=(tk == 0),
                                stop=(tk == TT - 1),
                            )
                        recip = sb.tile([1, SB], f32, tag="recip", bufs=2)
                        nc.vector.tensor_copy(recip[:], av[64:65, :])
                        bc = ps.tile([64, SB], f32, tag="sm", name="bc")
                        nc.tensor.matmul(
                            bc[:], (ones64[:]), (recip[:]), start=True, stop=True
                        )
                        bcs = sb.tile([64, SB], f32, tag="bcs", bufs=2, name="bcs")
                        nc.vector.tensor_copy(bcs[:], bc[:])
                        nc.vector.reciprocal(bcs[:], bcs[:])
                        dst = headout[p][
                            prow : prow + 64, sh * SQ + s2 * SB : sh * SQ + (s2 + 1) * SB
                        ]
                        nc.vector.tensor_mul(dst, av[0:64, :], bcs[:])
                        nc.vector.tensor_scalar_add(
                            dst, dst, bq_sb[prow : prow + 64, p : p + 1]
                        )

            # --- phase G: AllGather head outputs within batch group ---
            cc_in = dram.tile([NPAIR * 128, S], f32, name="cc_in")
            for p in range(NPAIR):
                nc.sync.dma_start(cc_in[p * 128 : (p + 1) * 128, :], headout[p][:])
            cc_out = dram.tile([HDK, S], f32, name="cc_out")
            nc.gpsimd.collective_compute(
                "AllGather",
                mybir.AluOpType.bypass,
                replica_groups=[[0, 1, 2, 3], [4, 5, 6, 7]],
                ins=[cc_in.opt()],
                outs=[cc_out.opt()],
            )

            # --- phase O: output projection (d-slice) ---
            for si in range(OSB):
                po = ps.tile([128, DSL], f32, tag="big", name="po")
                for k in range(KO):
                    ch = sb.tile([128, 128], fr, tag="ch", bufs=3, name="ch")
                    nc.sync.dma_start(
                        ch[:],
                        cc_out[
                            k * 128 : (k + 1) * 128, si * 128 : (si + 1) * 128
                        ].bitcast(fr),
                    )
                    nc.tensor.matmul(
                        po[:],
                        (ch[:]),
                        (woT_sb[:, k * DSL : (k + 1) * DSL]),
                        start=(k == 0),
                        stop=(k == KO - 1),
                    )
                ob = sb.tile([128, DSL], f32, tag="ob", bufs=3, name="ob")
                nc.vector.tensor_add(ob[:], po[:], bo_sb[:])
                nc.sync.dma_start(out_ap[si * 128 : (si + 1) * 128, :], ob[:])

    if split_waits:
        _split_multi_waits(nc)
    return nc


def make_in_maps(v, k, q, Wq, bqv, Wk, bkv, Wv, bvv, Wo, bov, S, DM, H, DK):
    """Per-core input dicts from full inputs (all host-side prep is slicing /
    transpose / trivial broadcast)."""
    HDK = H * DK
    DSL = HDK // GROUP
    xT = {}
    for b in range(2):
        xT[("q", b)] = np.ascontiguousarray(q[b].T)  # [DM, S]
        xT[("k", b)] = np.ascontiguousarray(k[b].T)
        xT[("v", b)] = np.ascontiguousarray(v[b].T)
    WoT = np.ascontiguousarray(Wo.T)  # [HDK, HDK_out]
    ident = np.vstack([np.eye(64, dtype=np.float32)] * 2)
    in_maps = []
    for c in range(N_CORES):
        b = c // GROUP
        h0 = 4 * (c % GROUP)
        m = {
            "xqT": xT[("q", b)],
            "xkT": xT[("k", b)],
            "xvT": xT[("v", b)],
            "ident": ident,
        }
        for kind, Wt, bt in (("q", Wq, bqv), ("k", Wk, bkv), ("v", Wv, bvv)):
            wp = np.empty((2, DM, 128), np.float32)
            bp = np.empty((2, 128, 1), np.float32)
            for p in range(2):
                ha, hb = h0 + 2 * p, h0 + 2 * p + 1
                wp[p, :, :64] = Wt[ha].T
                wp[p, :, 64:] = Wt[hb].T
                bp[p, :64, 0] = bt[ha]
                bp[p, 64:, 0] = bt[hb]
            m[f"w{kind}"] = wp
            if kind == "q":
                m["bq"] = bp
            elif kind == "k":
                m["bk8"] = bp / 8.0
            else:
                m["bv"] = bp
        d0 = DSL * (c % GROUP)
        m["woT"] = np.ascontiguousarray(WoT[:, d0 : d0 + DSL])
        m["bo_bc"] = np.ascontiguousarray(
            np.broadcast_to(bov[d0 : d0 + DSL], (128, DSL))
        )
        in_maps.append(m)
    return in_maps


def kernel(v, k, q, Wq, bq, Wk, bk, Wv, bv, Wo, bo, _trace=False):
    """Full inputs in, full output out. Runs the SPMD Bass kernel on 8 cores."""
    global last_results
    from concourse.bass_utils import run_bass_kernel_spmd

    v, k, q = (np.asarray(a, np.float32) for a in (v, k, q))
    B, S, DM = q.shape
    H, DK = Wq.shape[0], Wq.shape[1]
    HDK = H * DK
    DSL = HDK // GROUP

    nc = build_program(S=S, DM=DM, H=H, DK=DK)
    in_maps = make_in_maps(
        np.asarray(v, np.float32),
        np.asarray(k, np.float32),
        np.asarray(q, np.float32),
        *(np.asarray(a, np.float32) for a in (Wq, bq, Wk, bk, Wv, bv, Wo, bo)),
        S=S,
        DM=DM,
        H=H,
        DK=DK,
    )
    res = run_bass_kernel_spmd(
        nc, in_maps, list(range(N_CORES)), trace=_trace
    )
    last_results = res
    out = np.empty((B, S, HDK), np.float32)
    for c in range(N_CORES):
        b = c // GROUP
        d0 = DSL * (c % GROUP)
        out[b, :, d0 : d0 + DSL] = res.results[c]["out"]
    return out
